# revision 1
# baseline (speedup 1.0000x reference)
import numpy as np

STACK, UNITS, D, EPS = 12, 4, 128, 1e-3
NPART = 128


def prep_consts(inputs, T):
    """Host-side weight preprocessing. Returns dict of np arrays (all float32)."""
    ws = [np.asarray(inputs[f"w{i}"], np.float32) for i in range(STACK)]
    gamma = np.asarray(inputs["gamma"], np.float32)
    beta = np.asarray(inputs["beta"], np.float32)
    mean = np.asarray(inputs["mean"], np.float32)
    var = np.asarray(inputs["var"], np.float32)
    wf = np.asarray(inputs["wf"], np.float32)
    bf = np.asarray(inputs["bf"], np.float32)

    s = gamma / np.sqrt(var + EPS)           # [12, 4]
    bsh = beta - mean * s                    # [12, 4]
    wd = wf[:, 0] - wf[:, 1]                 # [176]
    bd = float(bf[0] - bf[1])

    c = {}
    # Wx [128, 49], columns u-major: col 12u+i = stage-i unit-u x-weights.
    # (u-major keeps the per-(t,u) cxF gather DMAs partition-contiguous.)
    Wx = np.zeros((D, 49), np.float32)
    for i in range(STACK):
        for u in range(UNITS):
            Wx[:, 12 * u + i] = ws[i][4 * i:, u] * s[i, u]
    Wx[:, 48] = wd[48:]
    c["wx"] = Wx

    # A_pack chunks: per stage i>=1, chunk c: [rows_c, 4T]
    BL = 4 * T  # stage block rows
    for i in range(1, STACK):
        K = BL * i
        for ci in range((K + NPART - 1) // NPART):
            rows = min(NPART, K - NPART * ci)
            M = np.zeros((rows, BL), np.float32)
            for r in range(rows):
                vr = NPART * ci + r
                j, rem = divmod(vr, BL)
                t, v = divmod(rem, 4)
                # Ahat_i[4j+v, u] = w_i[4(i-1-j)+v, u] * s[i,u]
                M[r, 4 * t:4 * t + 4] = ws[i][4 * (i - 1 - j) + v, :] * s[i]
            c[f"a_{i}_{ci}"] = M

    # wd_pack chunks: [rows_c, 4T] (columns replicated over u so output rows are
    # 4t+u, keeping every compute op 32-partition aligned)
    Kf = BL * STACK
    for ci in range((Kf + NPART - 1) // NPART):
        rows = min(NPART, Kf - NPART * ci)
        M = np.zeros((rows, BL), np.float32)
        for r in range(rows):
            vr = NPART * ci + r
            j, rem = divmod(vr, BL)
            t, v = divmod(rem, 4)
            M[r, 4 * t:4 * t + 4] = wd[4 * (11 - j) + v]
        c[f"wd_{ci}"] = M

    # bias_stt [4T, 12]: row 4t+u, col i = bsh[i, u]
    Bm = np.zeros((BL, STACK), np.float32)
    for i in range(STACK):
        Bm[:, i] = np.tile(bsh[i], T)
    c["bias_stt"] = Bm
    c["ident"] = np.eye(NPART, dtype=np.float32)
    c["bd"] = bd
    return c


def numpy_ref(x, consts, T):
    """Reference for the packed kernel (for sim checks). x [N, 128] -> [N, 2]."""
    xT = x.T.astype(np.float32)
    Cx = consts["wx"].T @ xT                             # [49, N], rows 12u+i
    N = x.shape[0]
    Y = np.zeros((48, N), np.float32)
    bias = consts["bias_stt"][:4, :]                     # [4, 12] (u, i)
    for i in range(STACK):
        z = Cx[[12 * u + i for u in range(4)], :] + bias[:, i][:, None]
        if i > 0:
            # rebuild Ahat from packs (t=0 block)
            acc = np.zeros((4, N), np.float32)
            for ci in range(100):
                key = f"a_{i}_{ci}"
                if key not in consts:
                    break
                M = consts[key]
                for r in range(M.shape[0]):
                    vr = 128 * ci + r
                    j, rem = divmod(vr, 4 * T)
                    t, v = divmod(rem, 4)
                    if t == 0:
                        acc += M[r, 0:4][:, None] * Y[4 * j + v][None, :]
            z = z + acc
        Y[4 * i:4 * i + 4] = np.maximum(z, 0.0)
    wdy = np.zeros(48, np.float32)
    for ci in range(3):
        M = consts[f"wd_{ci}"]
        for r in range(M.shape[0]):
            vr = 128 * ci + r
            j, rem = divmod(vr, 4 * T)
            t, v = divmod(rem, 4)
            if t == 0:
                wdy[4 * j + v] = M[r, 0]
    d = Cx[48] + wdy @ Y + consts["bd"]
    p0 = 1.0 / (1.0 + np.exp(-d))
    return np.stack([p0, 1.0 - p0], axis=-1).astype(np.float32)


def build_kernel(ctx, tc, outs, ins, *, T, NB, groups, bd):
    """Emit the kernel IR. ins/outs: dicts of bass APs.

    ins: x [groups*T*NB, 128], wx [128,49], a_{i}_{ci}, wd_{ci}, bias_stt [4T,12]
    outs: out [groups*T*NB, 2]
    """
    import concourse.bass as bass
    import concourse.mybir as mybir

    nc = tc.nc
    f32 = mybir.dt.float32
    f32r = mybir.dt.float32r
    ACT = mybir.ActivationFunctionType
    BL = 4 * T
    assert NB % 512 == 0
    NH = NB // 512  # psum-bank halves per tile column block
    n_chunks = (BL * STACK + NPART - 1) // NPART
    assert (BL * STACK) % NPART == 0, "chunk layout assumes full chunks"

    x_ap = ins["x"]
    out_ap = outs["out"]

    # ---- constant pools (loaded once) ----
    const_pool = ctx.enter_context(tc.tile_pool(name="consts", bufs=1))

    def load_const(name, shape, dt=f32):
        t = const_pool.tile(list(shape), dt, tag=name, name=name)
        nc.sync.dma_start(t[:], ins[name].bitcast(dt))
        return t

    ident = load_const("ident", (NPART, NPART), f32r)

    wx_sb = load_const("wx", (D, 49), f32r)
    a_sb = {}
    for i in range(1, STACK):
        K = BL * i
        for ci in range((K + NPART - 1) // NPART):
            rows = min(NPART, K - NPART * ci)
            a_sb[(i, ci)] = load_const(f"a_{i}_{ci}", (rows, BL), f32r)
    wd_sb = [load_const(f"wd_{ci}", (NPART, BL), f32r) for ci in range(n_chunks)]
    bias_sb = load_const("bias_stt", (BL, STACK))

    # ---- working pools ----
    xn_pool = ctx.enter_context(tc.tile_pool(name="xn", bufs=2))
    xt_pool = ctx.enter_context(tc.tile_pool(name="xt", bufs=2))
    cx_pool = ctx.enter_context(tc.tile_pool(name="cx", bufs=4))
    cxf_pool = ctx.enter_context(tc.tile_pool(name="cxf", bufs=1))
    y2_pool = ctx.enter_context(tc.tile_pool(name="y2", bufs=2))
    z_pool = ctx.enter_context(tc.tile_pool(name="z", bufs=2))
    dz_pool = ctx.enter_context(tc.tile_pool(name="dz", bufs=2))
    out_pool = ctx.enter_context(tc.tile_pool(name="outsb", bufs=2))

    pT_pool = ctx.enter_context(tc.tile_pool(name="pT", bufs=1, space="PSUM"))
    pCx_pool = ctx.enter_context(tc.tile_pool(name="pCx", bufs=1, space="PSUM"))
    pRec_pool = ctx.enter_context(tc.tile_pool(name="pRec", bufs=1, space="PSUM"))
    pD_pool = ctx.enter_context(tc.tile_pool(name="pD", bufs=1, space="PSUM"))

    rows_per_group = T * NB

    for g in range(groups):
        # cxF[4t+u, i*NB+n] = Cx_t[4i+u, n]; d-part at free block 12: cxF[4t+u, 12NB+n] = Cx_t[48, n]
        cxF = cxf_pool.tile([BL, (STACK + 1) * NB], f32, tag="cxF", name="cxF")
        # d-block rows 4t+u (u>0) are never written by the evac DMAs; zero them
        # so the (redundant) dz lanes stay finite
        nc.vector.memset(cxF[:, STACK * NB:(STACK + 1) * NB], 0.0)
        y2 = [y2_pool.tile([NPART, NB], f32r, tag=f"y2c{c}", name=f"y2c{c}") for c in range(n_chunks)]
        for t in range(T):
            r0 = g * rows_per_group + t * NB
            # natural-layout load: [NB, 128] rows -> sbuf [128, NB] as (c p) f -> p c f
            xn = xn_pool.tile([NPART, NB], f32r, tag="xn")
            src = x_ap.bitcast(f32r)[r0:r0 + NB, :].rearrange("(c p) f -> p c f", p=NPART)
            dst3 = xn[:].rearrange("p (c f) -> p c f", f=D)
            nc.sync.dma_start(dst3, src)
            # PE transpose per 128-block
            pT = pT_pool.tile([NPART, NB], f32r, tag="pT")
            nblk = NB // NPART
            for b in range(nblk):
                nc.tensor.transpose(
                    pT[:, b * NPART:(b + 1) * NPART],
                    xn[:, b * NPART:(b + 1) * NPART],
                    ident[:],
                )
            xt = xt_pool.tile([NPART, NB], f32r, tag="xt")
            nc.scalar.activation(xt[:], pT[:].bitcast(f32), ACT.Copy)
            # x-matmul -> Cx [49, NB] psum
            pcx = pCx_pool.tile([49, NB], f32, tag="pCx")
            for h in range(NH):
                sl = slice(h * 512, (h + 1) * 512)
                nc.tensor.matmul(
                    pcx[:, sl], wx_sb[:], xt[:, sl],
                    start=True, stop=True,
                )
            # psum -> sbuf (ACT, aligned), then sbuf->sbuf DMA gather into
            # interleaved cxF (DMA has no partition-start limits)
            cx = cx_pool.tile([49, NB], f32, tag="cx")
            nc.scalar.activation(cx[:], pcx[:], ACT.Copy)
            for u in range(4):
                esrc = cx[12 * u:12 * u + 12, :]           # contiguous partitions
                edst = cxF[4 * t + u:4 * t + u + 1, 0:STACK * NB].rearrange(
                    "p (i n) -> p i n", i=STACK)
                nc.sync.dma_start(edst, esrc)
            nc.sync.dma_start(
                cxF[4 * t:4 * t + 1, STACK * NB:(STACK + 1) * NB],
                cx[48:49, :],
            )

        # stage 0: y_0 = relu(cxF[:, 0:NB] + bsh_0)
        nc.scalar.activation(
            y2[0][0:BL, :], cxF[:, 0:NB], ACT.Relu, bias=bias_sb[:, 0:1],
        )

        # recurrence
        for i in range(1, STACK):
            K = BL * i
            prec = pRec_pool.tile([BL, NB], f32, tag="pRec")
            ncch = (K + NPART - 1) // NPART
            for h in range(NH):
                sl = slice(h * 512, (h + 1) * 512)
                for ci in range(ncch):
                    rows = min(NPART, K - NPART * ci)
                    nc.tensor.matmul(
                        prec[:, sl],
                        a_sb[(i, ci)][:],
                        y2[ci][0:rows, sl],
                        start=(ci == 0), stop=(ci == ncch - 1),
                    )
            z = z_pool.tile([BL, NB], f32, tag="z")
            nc.vector.scalar_tensor_tensor(
                z[:], prec[:], bias_sb[:, i:i + 1],
                cxF[:, i * NB:(i + 1) * NB],
                mybir.AluOpType.add, mybir.AluOpType.add,
            )
            ch, ro = divmod(BL * i, NPART)
            nc.scalar.activation(y2[ch][ro:ro + BL, :], z[:], ACT.Relu)

        # final d (rows 4t+u all carry d_t; redundant over u)
        pd = pD_pool.tile([BL, NB], f32, tag="pD")
        for h in range(NH):
            sl = slice(h * 512, (h + 1) * 512)
            for ci in range(n_chunks):
                nc.tensor.matmul(
                    pd[:, sl], wd_sb[ci][:], y2[ci][:, sl],
                    start=(ci == 0), stop=(ci == n_chunks - 1),
                )
        dz = dz_pool.tile([BL, NB], f32, tag="dz")
        nc.vector.scalar_tensor_tensor(
            dz[:], pd[:], float(bd), cxF[:, STACK * NB:(STACK + 1) * NB],
            mybir.AluOpType.add, mybir.AluOpType.add,
        )
        outsb = out_pool.tile([BL, 2 * NB], f32, tag="outsb")
        o3 = outsb[:].rearrange("p (n two) -> p n two", two=2)
        nc.scalar.activation(o3[:, :, 0], dz[:], ACT.Sigmoid)
        nc.scalar.activation(o3[:, :, 1], dz[:], ACT.Sigmoid, scale=-1.0)
        # out rows 4t carry tile t's [NB, 2] flattened; DMA gathers the stride-4 rows
        dst = out_ap[g * rows_per_group:(g + 1) * rows_per_group, :]
        osrc = outsb[:].rearrange("(t four) m -> t four m", four=4)[:, 0, :]
        nc.sync.dma_start(dst.rearrange("(t n) two -> t (n two)", t=T), osrc)


# ---------------------------------------------------------------------------
# Self-contained entry point: kernel(**inputs) -> [500000, 2] float32
# ---------------------------------------------------------------------------

import sys as _sys
if '/opt/trn_rl_repo' not in _sys.path:
    _sys.path.insert(0, '/opt/trn_rl_repo')

B_FULL = 500000
N_CORES = 8
T_CFG = 8
NB_CFG = 1024
GROUPS_CFG = 8
CORE_ROWS = T_CFG * NB_CFG * GROUPS_CFG          # 65536
B_PAD = CORE_ROWS * N_CORES                      # 524288

_CACHE = {}


def _build_nc(const_shapes, bd):
    from contextlib import ExitStack
    import concourse.mybir as mybir
    from concourse import bacc
    import concourse.tile as tile

    nc = bacc.Bacc("TRN2", target_bir_lowering=False, debug=False,
                   num_devices=N_CORES)
    ins = {}
    ins["x"] = nc.dram_tensor("x", [CORE_ROWS, D], mybir.dt.float32,
                              kind="ExternalInput").ap()
    for name, shp in const_shapes.items():
        ins[name] = nc.dram_tensor(name, list(shp), mybir.dt.float32,
                                   kind="ExternalInput").ap()
    outs = {"out": nc.dram_tensor("out", [CORE_ROWS, 2], mybir.dt.float32,
                                  kind="ExternalOutput").ap()}
    with tile.TileContext(nc) as tc:
        with ExitStack() as ctx:
            build_kernel(ctx, tc, outs, ins, T=T_CFG, NB=NB_CFG,
                         groups=GROUPS_CFG, bd=bd)
    nc.compile()
    return nc


def kernel(**inputs):
    import numpy as np
    from concourse.bass_utils import run_bass_kernel_spmd

    consts = prep_consts(inputs, T_CFG)
    bd = consts.pop("bd")
    x = np.ascontiguousarray(np.asarray(inputs["x"], dtype=np.float32))
    assert x.shape == (B_FULL, D)
    xp = np.zeros((B_PAD, D), np.float32)
    xp[:B_FULL] = x

    key = "nc"
    if key not in _CACHE:
        _CACHE[key] = _build_nc({k: v.shape for k, v in consts.items()}, bd)
    nc = _CACHE[key]

    in_maps = []
    for c in range(N_CORES):
        m = {"x": xp[c * CORE_ROWS:(c + 1) * CORE_ROWS]}
        m.update(consts)
        in_maps.append(m)
    res = run_bass_kernel_spmd(nc, in_maps, core_ids=list(range(N_CORES)))
    out = np.concatenate([res.results[c]["out"] for c in range(N_CORES)], axis=0)
    return out[:B_FULL]



# revision 2
# speedup vs baseline: 1.0031x; 1.0031x over previous
import numpy as np

STACK, UNITS, D, EPS = 12, 4, 128, 1e-3
NPART = 128


def _bf16(a):
    import ml_dtypes
    return np.asarray(a, dtype=ml_dtypes.bfloat16)


def prep_consts(inputs, T):
    """Host-side weight preprocessing. Returns dict of np arrays."""
    ws = [np.asarray(inputs[f"w{i}"], np.float32) for i in range(STACK)]
    gamma = np.asarray(inputs["gamma"], np.float32)
    beta = np.asarray(inputs["beta"], np.float32)
    mean = np.asarray(inputs["mean"], np.float32)
    var = np.asarray(inputs["var"], np.float32)
    wf = np.asarray(inputs["wf"], np.float32)
    bf = np.asarray(inputs["bf"], np.float32)

    s = gamma / np.sqrt(var + EPS)           # [12, 4]
    bsh = beta - mean * s                    # [12, 4]
    wd = wf[:, 0] - wf[:, 1]                 # [176]
    bd = float(bf[0] - bf[1])

    c = {}
    # Wx [128, 49], columns u-major: col 12u+i = stage-i unit-u x-weights.
    Wx = np.zeros((D, 49), np.float32)
    for i in range(STACK):
        for u in range(UNITS):
            Wx[:, 12 * u + i] = ws[i][4 * i:, u] * s[i, u]
    Wx[:, 48] = wd[48:]
    c["wx"] = _bf16(Wx)

    # A_pack chunks: per stage i>=1, chunk c: [rows_c, 4T]
    BL = 4 * T  # stage block rows
    for i in range(1, STACK):
        K = BL * i
        for ci in range((K + NPART - 1) // NPART):
            rows = min(NPART, K - NPART * ci)
            M = np.zeros((rows, BL), np.float32)
            for r in range(rows):
                vr = NPART * ci + r
                j, rem = divmod(vr, BL)
                t, v = divmod(rem, 4)
                # Ahat_i[4j+v, u] = w_i[4(i-1-j)+v, u] * s[i,u]
                M[r, 4 * t:4 * t + 4] = ws[i][4 * (i - 1 - j) + v, :] * s[i]
            c[f"a_{i}_{ci}"] = M

    # wd_pack chunks: [rows_c, 4T] (columns replicated over u so output rows are
    # 4t+u, keeping every compute op 32-partition aligned)
    Kf = BL * STACK
    for ci in range((Kf + NPART - 1) // NPART):
        rows = min(NPART, Kf - NPART * ci)
        M = np.zeros((rows, BL), np.float32)
        for r in range(rows):
            vr = NPART * ci + r
            j, rem = divmod(vr, BL)
            t, v = divmod(rem, 4)
            M[r, 4 * t:4 * t + 4] = wd[4 * (11 - j) + v]
        c[f"wd_{ci}"] = M

    # bias_stt [4T, 12]: row 4t+u, col i = bsh[i, u]
    Bm = np.zeros((BL, STACK), np.float32)
    for i in range(STACK):
        Bm[:, i] = np.tile(bsh[i], T)
    c["bias_stt"] = Bm
    c["bd"] = bd
    return c


def build_kernel(ctx, tc, outs, ins, *, T, NB, groups, bd):
    """Emit the kernel IR. ins/outs: dicts of bass APs.

    ins: x [groups*T*NB, 128] bf16, wx [128,49] bf16, a_{i}_{ci}, wd_{ci},
         bias_stt [4T,12]
    outs: out [groups*T*NB, 2]
    """
    import concourse.bass as bass
    import concourse.mybir as mybir

    nc = tc.nc
    f32 = mybir.dt.float32
    f32r = mybir.dt.float32r
    bf16 = mybir.dt.bfloat16
    ACT = mybir.ActivationFunctionType
    BL = 4 * T
    assert NB % 512 == 0
    NH = NB // 512  # psum-bank halves per tile column block
    n_chunks = (BL * STACK + NPART - 1) // NPART
    assert (BL * STACK) % NPART == 0, "chunk layout assumes full chunks"

    x_ap = ins["x"]
    out_ap = outs["out"]

    # ---- constant pools (loaded once) ----
    const_pool = ctx.enter_context(tc.tile_pool(name="consts", bufs=1))

    def load_const(name, shape, dt=f32):
        t = const_pool.tile(list(shape), dt, tag=name, name=name)
        if dt == f32r:
            nc.sync.dma_start(t[:], ins[name].bitcast(dt))
        else:
            nc.sync.dma_start(t[:], ins[name])
        return t

    wx_sb = load_const("wx", (D, 49), bf16)
    a_sb = {}
    for i in range(1, STACK):
        K = BL * i
        for ci in range((K + NPART - 1) // NPART):
            rows = min(NPART, K - NPART * ci)
            a_sb[(i, ci)] = load_const(f"a_{i}_{ci}", (rows, BL), f32r)
    wd_sb = [load_const(f"wd_{ci}", (NPART, BL), f32r) for ci in range(n_chunks)]
    bias_sb = load_const("bias_stt", (BL, STACK))

    # ---- working pools ----
    xt_pool = ctx.enter_context(tc.tile_pool(name="xt", bufs=3))
    cx_pool = ctx.enter_context(tc.tile_pool(name="cx", bufs=4))
    cxf_pool = ctx.enter_context(tc.tile_pool(name="cxf", bufs=1))
    y2_pool = ctx.enter_context(tc.tile_pool(name="y2", bufs=2))
    z_pool = ctx.enter_context(tc.tile_pool(name="z", bufs=2))
    dz_pool = ctx.enter_context(tc.tile_pool(name="dz", bufs=2))
    out_pool = ctx.enter_context(tc.tile_pool(name="outsb", bufs=2))

    pCx_pool = ctx.enter_context(tc.tile_pool(name="pCx", bufs=2, space="PSUM"))
    pRec_pool = ctx.enter_context(tc.tile_pool(name="pRec", bufs=1, space="PSUM"))
    pD_pool = ctx.enter_context(tc.tile_pool(name="pD", bufs=1, space="PSUM"))

    rows_per_group = T * NB

    for g in range(groups):
        # cxF[4t+u, i*NB+n] = Cx_t[4i+u, n]; d-part at free block 12
        cxF = cxf_pool.tile([BL, (STACK + 1) * NB], f32, tag="cxF", name="cxF")
        y2 = [y2_pool.tile([NPART, NB], f32r, tag=f"y2c{c}", name=f"y2c{c}") for c in range(n_chunks)]
        for t in range(T):
            r0 = g * rows_per_group + t * NB
            # xbar DMA transpose: [NB, 128] bf16 rows -> sbuf [128, NB]
            xt = xt_pool.tile([NPART, NB], bf16, tag="xt")
            nc.sync.dma_start(xt[:], x_ap[r0:r0 + NB, :], transpose=True)
            # x-matmul -> Cx [49, NB] psum
            pcx = pCx_pool.tile([49, NB], f32, tag="pCx")
            for h in range(NH):
                sl = slice(h * 512, (h + 1) * 512)
                nc.tensor.matmul(
                    pcx[:, sl], wx_sb[:], xt[:, sl],
                    start=True, stop=True,
                )
            # psum -> sbuf (ACT, aligned), then sbuf->sbuf DMA gather into
            # interleaved cxF
            cx = cx_pool.tile([49, NB], f32, tag="cx")
            nc.scalar.activation(cx[:], pcx[:], ACT.Copy)
            for u in range(4):
                esrc = cx[12 * u:12 * u + 12, :]           # contiguous partitions
                edst = cxF[4 * t + u:4 * t + u + 1, 0:STACK * NB].rearrange(
                    "p (i n) -> p i n", i=STACK)
                nc.sync.dma_start(edst, esrc)
            nc.sync.dma_start(
                cxF[4 * t:4 * t + 1, STACK * NB:(STACK + 1) * NB],
                cx[48:49, :],
            )

        # stage 0: y_0 = relu(cxF[:, 0:NB] + bsh_0)
        nc.scalar.activation(
            y2[0][0:BL, :], cxF[:, 0:NB], ACT.Relu, bias=bias_sb[:, 0:1],
        )

        # recurrence
        for i in range(1, STACK):
            K = BL * i
            prec = pRec_pool.tile([BL, NB], f32, tag="pRec")
            ncch = (K + NPART - 1) // NPART
            for h in range(NH):
                sl = slice(h * 512, (h + 1) * 512)
                for ci in range(ncch):
                    rows = min(NPART, K - NPART * ci)
                    nc.tensor.matmul(
                        prec[:, sl],
                        a_sb[(i, ci)][:],
                        y2[ci][0:rows, sl],
                        start=(ci == 0), stop=(ci == ncch - 1),
                    )
            z = z_pool.tile([BL, NB], f32, tag="z")
            nc.vector.scalar_tensor_tensor(
                z[:], prec[:], bias_sb[:, i:i + 1],
                cxF[:, i * NB:(i + 1) * NB],
                mybir.AluOpType.add, mybir.AluOpType.add,
            )
            ch, ro = divmod(BL * i, NPART)
            nc.scalar.activation(y2[ch][ro:ro + BL, :], z[:], ACT.Relu)

        # final d (rows 4t+u all carry d_t; redundant over u)
        pd = pD_pool.tile([BL, NB], f32, tag="pD")
        for h in range(NH):
            sl = slice(h * 512, (h + 1) * 512)
            for ci in range(n_chunks):
                nc.tensor.matmul(
                    pd[:, sl], wd_sb[ci][:], y2[ci][:, sl],
                    start=(ci == 0), stop=(ci == n_chunks - 1),
                )
        dz = dz_pool.tile([BL, NB], f32, tag="dz")
        nc.vector.scalar_tensor_tensor(
            dz[:], pd[:], float(bd), cxF[:, STACK * NB:(STACK + 1) * NB],
            mybir.AluOpType.add, mybir.AluOpType.add,
        )
        outsb = out_pool.tile([BL, 2 * NB], f32, tag="outsb")
        o3 = outsb[:].rearrange("p (n two) -> p n two", two=2)
        nc.scalar.activation(o3[:, :, 0], dz[:], ACT.Sigmoid)
        nc.scalar.activation(o3[:, :, 1], dz[:], ACT.Sigmoid, scale=-1.0)
        # out rows 4t carry tile t's [NB, 2] flattened
        dst = out_ap[g * rows_per_group:(g + 1) * rows_per_group, :]
        osrc = outsb[:].rearrange("(t four) m -> t four m", four=4)[:, 0, :]
        nc.sync.dma_start(dst.rearrange("(t n) two -> t (n two)", t=T), osrc)


# ---------------------------------------------------------------------------
# Self-contained entry point: kernel(**inputs) -> [500000, 2] float32
# ---------------------------------------------------------------------------

import sys as _sys
if '/opt/trn_rl_repo' not in _sys.path:
    _sys.path.insert(0, '/opt/trn_rl_repo')

B_FULL = 500000
N_CORES = 8
T_CFG = 8
NB_CFG = 1024
GROUPS_CFG = 8
CORE_ROWS = T_CFG * NB_CFG * GROUPS_CFG          # 65536
B_PAD = CORE_ROWS * N_CORES                      # 524288

_CACHE = {}


def _build_nc(const_shapes, bd):
    from contextlib import ExitStack
    import concourse.mybir as mybir
    from concourse import bacc
    import concourse.tile as tile

    nc = bacc.Bacc("TRN2", target_bir_lowering=False, debug=False,
                   num_devices=N_CORES)
    ins = {}
    ins["x"] = nc.dram_tensor("x", [CORE_ROWS, D], mybir.dt.bfloat16,
                              kind="ExternalInput").ap()
    for name, shp, npdt in const_shapes:
        dt = mybir.dt.bfloat16 if npdt == 'bfloat16' else mybir.dt.float32
        ins[name] = nc.dram_tensor(name, list(shp), dt,
                                   kind="ExternalInput").ap()
    outs = {"out": nc.dram_tensor("out", [CORE_ROWS, 2], mybir.dt.float32,
                                  kind="ExternalOutput").ap()}
    with tile.TileContext(nc) as tc:
        with ExitStack() as ctx:
            build_kernel(ctx, tc, outs, ins, T=T_CFG, NB=NB_CFG,
                         groups=GROUPS_CFG, bd=bd)
    nc.compile()
    return nc


def kernel(**inputs):
    import numpy as np
    import ml_dtypes
    from concourse.bass_utils import run_bass_kernel_spmd

    consts = prep_consts(inputs, T_CFG)
    bd = consts.pop("bd")
    x = np.asarray(inputs["x"], dtype=np.float32)
    assert x.shape == (B_FULL, D)
    xp = np.zeros((B_PAD, D), ml_dtypes.bfloat16)
    xp[:B_FULL] = x.astype(ml_dtypes.bfloat16)

    key = "nc"
    if key not in _CACHE:
        shapes = tuple((k, v.shape, str(v.dtype)) for k, v in consts.items())
        _CACHE[key] = _build_nc(shapes, bd)
    nc = _CACHE[key]

    in_maps = []
    for c in range(N_CORES):
        m = {"x": xp[c * CORE_ROWS:(c + 1) * CORE_ROWS]}
        m.update(consts)
        in_maps.append(m)
    res = run_bass_kernel_spmd(nc, in_maps, core_ids=list(range(N_CORES)))
    out = np.concatenate([res.results[c]["out"] for c in range(N_CORES)], axis=0)
    return out[:B_FULL]


# revision 12
# speedup vs baseline: 1.9359x; 1.9299x over previous
import numpy as np

STACK, UNITS, D, EPS = 12, 4, 128, 1e-3
NPART = 128
T, NB = 16, 2048                 # t-blocks per group, cols per t-block
GROUPS = 2
G_ROWS = T * NB                  # 32768
NH = NB // 1024                  # rc rounds per stage (1024-col z tiles)


def _bf16(a):
    import ml_dtypes
    return np.asarray(a, dtype=ml_dtypes.bfloat16)


def prep_consts(inputs):
    """Host-side weight packing for the u-major T=16 layout."""
    ws = [np.asarray(inputs[f"w{i}"], np.float32) for i in range(STACK)]
    gamma = np.asarray(inputs["gamma"], np.float32)
    beta = np.asarray(inputs["beta"], np.float32)
    mean = np.asarray(inputs["mean"], np.float32)
    var = np.asarray(inputs["var"], np.float32)
    wf = np.asarray(inputs["wf"], np.float32)
    bf = np.asarray(inputs["bf"], np.float32)

    s = gamma / np.sqrt(var + EPS)
    bsh = beta - mean * s
    wd = wf[:, 0] - wf[:, 1]
    bd = float(bf[0] - bf[1])

    c = {}
    Wx = np.zeros((D, 49), np.float32)
    for i in range(STACK):
        for u in range(UNITS):
            Wx[:, 12 * u + i] = ws[i][4 * i:, u] * s[i, u]
    Wx[:, 48] = wd[48:]
    c["wx"] = _bf16(Wx)

    # A chunks: stage i, chunk cc = source stages {2cc, 2cc+1}
    for i in range(1, STACK):
        for cc in range(i // 2):
            M = np.zeros((128, 64), np.float32)
            for jj in range(2):
                j = 2 * cc + jj
                for v in range(4):
                    for u in range(4):
                        val = ws[i][4 * (i - 1 - j) + v, u] * s[i, u]
                        M[64 * jj + 16 * v:64 * jj + 16 * v + 16, 16 * u:16 * u + 16] \
                            [np.arange(16), np.arange(16)] = val
            c[f"a_{i}_{cc}"] = _bf16(M)
        if i % 2 == 1:
            j = i - 1
            M = np.zeros((64, 64), np.float32)
            for v in range(4):
                for u in range(4):
                    val = ws[i][4 * (i - 1 - j) + v, u] * s[i, u]
                    M[16 * v:16 * v + 16, 16 * u:16 * u + 16][np.arange(16), np.arange(16)] = val
            c[f"ap_{i}"] = _bf16(M)

    for cc in range(6):
        M = np.zeros((128, 64), np.float32)
        for jj in range(2):
            j = 2 * cc + jj
            for v in range(4):
                val = wd[4 * (11 - j) + v]
                for u in range(4):
                    M[64 * jj + 16 * v:64 * jj + 16 * v + 16, 16 * u:16 * u + 16] \
                        [np.arange(16), np.arange(16)] = val
        c[f"wd_{cc}"] = _bf16(M)

    c["s_id"] = _bf16(np.eye(64, dtype=np.float32))
    S_d = np.zeros((64, 64), np.float32)
    for t in range(T):
        for u in range(4):
            S_d[t, 16 * u + t] = 1.0
    c["s_d"] = _bf16(S_d)

    B = np.zeros((64, STACK), np.float32)
    for i in range(STACK):
        for u in range(4):
            B[16 * u:16 * u + 16, i] = bsh[i, u]
    c["bias"] = B
    c["bd"] = bd
    return c


def build_kernel(ctx, tc, outs, ins, *, bd):
    import concourse.mybir as mybir

    nc = tc.nc
    f32 = mybir.dt.float32
    bf16 = mybir.dt.bfloat16
    ACT = mybir.ActivationFunctionType
    ALU = mybir.AluOpType

    x_ap = ins["x"]
    out_ap = outs["out"]

    const_pool = ctx.enter_context(tc.tile_pool(name="consts", bufs=1))

    def load_const(name, shape, dt=f32):
        t = const_pool.tile(list(shape), dt, tag=name, name=name)
        nc.sync.dma_start(t[:], ins[name])
        return t

    wx_sb = load_const("wx", (D, 49), bf16)
    a_sb = {}
    ap_sb = {}
    for i in range(1, STACK):
        for cc in range(i // 2):
            a_sb[(i, cc)] = load_const(f"a_{i}_{cc}", (128, 64), bf16)
        if i % 2 == 1:
            ap_sb[i] = load_const(f"ap_{i}", (64, 64), bf16)
    wd_sb = [load_const(f"wd_{cc}", (128, 64), bf16) for cc in range(6)]
    sid_sb = load_const("s_id", (64, 64), bf16)
    sd_sb = load_const("s_d", (64, 64), bf16)
    bias_sb = load_const("bias", (64, STACK))

    xt_pool = ctx.enter_context(tc.tile_pool(name="xt", bufs=3))
    cx_pool = ctx.enter_context(tc.tile_pool(name="cx", bufs=3))
    cxf_pool = ctx.enter_context(tc.tile_pool(name="cxf", bufs=2))
    cxfd_pool = ctx.enter_context(tc.tile_pool(name="cxfd", bufs=2))
    y2_pool = ctx.enter_context(tc.tile_pool(name="y2", bufs=2))
    out_pool = ctx.enter_context(tc.tile_pool(name="outsb", bufs=2))

    pcx_pool = ctx.enter_context(tc.tile_pool(name="pcx", bufs=2, space="PSUM"))
    z_pool = ctx.enter_context(tc.tile_pool(name="z", bufs=2, space="PSUM"))

    for g in range(GROUPS):
        cxF = cxf_pool.tile([64, STACK * NB], bf16, tag="cxF", name="cxF")
        cxFd = cxfd_pool.tile([64, NB], bf16, tag="cxFd", name="cxFd")
        # rows 16..64 of cxFd feed the K=64 d-inject; zero everything first
        # (scatters then overwrite rows 0..15; engine partition bases must be
        # 32-aligned so a [16:64] memset is not expressible)
        nc.vector.memset(cxFd[:], 0.0)
        y2 = [y2_pool.tile([128, NB], bf16, tag=f"y2c{cc}", name=f"y2c{cc}")
              for cc in range(6)]

        # ---- x phase: per t-block: transpose-load, x-matmul, evac, scatter ----
        for t in range(T):
            r0 = g * G_ROWS + t * NB
            xt = xt_pool.tile([NPART, NB], bf16, tag="xt")
            nc.sync.dma_start(xt[:], x_ap[r0:r0 + NB, :], transpose=True)
            cx = cx_pool.tile([49, NB], bf16, tag="cx")
            for rc in range(NH):
                pcx = pcx_pool.tile([49, 1024], f32, tag="pcx")
                for h in range(2):
                    nc.tensor.matmul(
                        pcx[:, h * 512:(h + 1) * 512], wx_sb[:],
                        xt[:, rc * 1024 + h * 512: rc * 1024 + (h + 1) * 512],
                        start=True, stop=True,
                    )
                dst = cx[:, rc * 1024:(rc + 1) * 1024]
                if (t + rc) % 2 == 0:
                    nc.scalar.activation(dst, pcx[:], ACT.Copy)
                else:
                    nc.vector.tensor_copy(dst, pcx[:])
            # one-DMA scatter: rows 12u+i -> cxF[16u+t, block i]
            ed = cxF[:].rearrange("(u s) (i n) -> u s i n", u=4, i=STACK)[:, t]
            nc.gpsimd.dma_start(ed, cx[0:48, :])
            nc.scalar.dma_start(cxFd[t:t + 1, :], cx[48:49, :])

        # ---- recurrence ----
        for i in range(STACK):
            z = z_pool.tile([128, 1024], f32, tag="z")
            for rc in range(NH):
                ncc = i // 2
                has_part = (i % 2 == 1)
                for cb in range(2):
                    zsl = z[cb * 64:(cb + 1) * 64, rc * 512:(rc + 1) * 512]
                    csl = slice(i * NB + rc * 1024 + cb * 512,
                                i * NB + rc * 1024 + (cb + 1) * 512)
                    nc.tensor.matmul(zsl, sid_sb[:], cxF[0:64, csl],
                                     start=True,
                                     stop=(ncc == 0 and not has_part))
                if has_part:
                    for cb in range(2):
                        zsl = z[cb * 64:(cb + 1) * 64, rc * 512:(rc + 1) * 512]
                        ysl = slice(rc * 1024 + cb * 512, rc * 1024 + (cb + 1) * 512)
                        nc.tensor.matmul(zsl, ap_sb[i][:], y2[i // 2][0:64, ysl],
                                         start=False, stop=(ncc == 0))
                for cc in range(ncc):
                    for cb in range(2):
                        zsl = z[cb * 64:(cb + 1) * 64, rc * 512:(rc + 1) * 512]
                        ysl = slice(rc * 1024 + cb * 512, rc * 1024 + (cb + 1) * 512)
                        nc.tensor.matmul(zsl, a_sb[(i, cc)][:], y2[cc][:, ysl],
                                         start=False, stop=(cc == ncc - 1))
            # relu + bias -> y2 slice (per cb; strided dst over rc)
            ch, half = i // 2, 64 * (i % 2)
            for cb in range(2):
                src = z[cb * 64:(cb + 1) * 64, :].rearrange("p (rc n) -> p rc n", rc=NH)
                dst = y2[ch][half:half + 64, :].rearrange(
                    "p (rc c n) -> p rc c n", rc=NH, c=2)[:, :, cb, :]
                if (i + cb) % 2 == 0:
                    nc.scalar.activation(dst, src, ACT.Relu, bias=bias_sb[:, i:i + 1])
                else:
                    nc.vector.tensor_scalar(dst, src, bias_sb[:, i:i + 1], 0.0,
                                            ALU.add, ALU.max)

        # ---- wd chain + sigmoid + out ----
        pd = z_pool.tile([128, 1024], f32, tag="z")
        for rc in range(NH):
            for cb in range(2):
                psl = pd[cb * 64:(cb + 1) * 64, rc * 512:(rc + 1) * 512]
                dsl = slice(rc * 1024 + cb * 512, rc * 1024 + (cb + 1) * 512)
                nc.tensor.matmul(psl, sd_sb[:], cxFd[0:64, dsl],
                                 start=True, stop=False)
            for cc in range(6):
                for cb in range(2):
                    psl = pd[cb * 64:(cb + 1) * 64, rc * 512:(rc + 1) * 512]
                    ysl = slice(rc * 1024 + cb * 512, rc * 1024 + (cb + 1) * 512)
                    nc.tensor.matmul(psl, wd_sb[cc][:], y2[cc][:, ysl],
                                     start=False, stop=(cc == 5))
        outsb = out_pool.tile([128, NB], f32, tag="outsb")
        o4 = outsb[:].rearrange("p (rc n two) -> p rc n two", rc=NH, two=2)
        psrc = pd[:].rearrange("p (rc n) -> p rc n", rc=NH)
        nc.scalar.activation(o4[:, :, :, 0], psrc, ACT.Sigmoid, bias=float(bd))
        nc.scalar.activation(o4[:, :, :, 1], psrc, ACT.Sigmoid, bias=float(-bd),
                             scale=-1.0)
        # out rows: t*NB + rc*1024 + cb*512 + n  <- outsb[cb*64 + t]
        og = out_ap[g * G_ROWS:(g + 1) * G_ROWS, :].rearrange(
            "(t rc c n) two -> c t rc (n two)", rc=NH, c=2, n=512)
        for cb in range(2):
            osrc = outsb[cb * 64:cb * 64 + T, :].rearrange("p (rc f) -> p rc f", rc=NH)
            nc.sync.dma_start(og[cb], osrc)


# ---------------------------------------------------------------------------
# Self-contained entry point: kernel(**inputs) -> [500000, 2] float32
# ---------------------------------------------------------------------------

import sys as _sys
if '/opt/trn_rl_repo' not in _sys.path:
    _sys.path.insert(0, '/opt/trn_rl_repo')

B_FULL = 500000
N_CORES = 8
CORE_ROWS = GROUPS * G_ROWS                      # 65536
B_PAD = CORE_ROWS * N_CORES                      # 524288

_CACHE = {}


def _build_nc(const_shapes, bd):
    from contextlib import ExitStack
    import concourse.mybir as mybir
    from concourse import bacc
    import concourse.tile as tile

    nc = bacc.Bacc("TRN2", target_bir_lowering=False, debug=False,
                   num_devices=N_CORES)
    ins = {}
    ins["x"] = nc.dram_tensor("x", [CORE_ROWS, D], mybir.dt.bfloat16,
                              kind="ExternalInput").ap()
    for name, shp, npdt in const_shapes:
        dt = mybir.dt.bfloat16 if npdt == 'bfloat16' else mybir.dt.float32
        ins[name] = nc.dram_tensor(name, list(shp), dt,
                                   kind="ExternalInput").ap()
    outs = {"out": nc.dram_tensor("out", [CORE_ROWS, 2], mybir.dt.float32,
                                  kind="ExternalOutput").ap()}
    with tile.TileContext(nc) as tc:
        with ExitStack() as ctx:
            build_kernel(ctx, tc, outs, ins, bd=bd)
    nc.compile()
    return nc


def kernel(**inputs):
    import numpy as np
    import ml_dtypes
    from concourse.bass_utils import run_bass_kernel_spmd

    consts = prep_consts(inputs)
    bd = consts.pop("bd")
    x = np.asarray(inputs["x"], dtype=np.float32)
    assert x.shape == (B_FULL, D)
    xp = np.zeros((B_PAD, D), ml_dtypes.bfloat16)
    xp[:B_FULL] = x.astype(ml_dtypes.bfloat16)

    key = "nc"
    if key not in _CACHE:
        shapes = tuple((k, v.shape, str(v.dtype)) for k, v in consts.items())
        _CACHE[key] = _build_nc(shapes, bd)
    nc = _CACHE[key]

    in_maps = []
    for c in range(N_CORES):
        m = {"x": xp[c * CORE_ROWS:(c + 1) * CORE_ROWS]}
        m.update(consts)
        in_maps.append(m)
    res = run_bass_kernel_spmd(nc, in_maps, core_ids=list(range(N_CORES)))
    out = np.concatenate([res.results[c]["out"] for c in range(N_CORES)], axis=0)
    return out[:B_FULL]


# revision 13
# speedup vs baseline: 2.3824x; 1.2306x over previous
import numpy as np

STACK, UNITS, D, EPS = 12, 4, 128, 1e-3
NPART = 128
T, NB = 16, 2048                 # t-blocks per group, cols per t-block
GROUPS = 2
G_ROWS = T * NB                  # 32768
NH = NB // 1024                  # rc rounds per stage (1024-col z tiles)


def _bf16(a):
    import ml_dtypes
    return np.asarray(a, dtype=ml_dtypes.bfloat16)


def prep_consts(inputs):
    """Host-side weight packing for the u-major T=16 layout."""
    ws = [np.asarray(inputs[f"w{i}"], np.float32) for i in range(STACK)]
    gamma = np.asarray(inputs["gamma"], np.float32)
    beta = np.asarray(inputs["beta"], np.float32)
    mean = np.asarray(inputs["mean"], np.float32)
    var = np.asarray(inputs["var"], np.float32)
    wf = np.asarray(inputs["wf"], np.float32)
    bf = np.asarray(inputs["bf"], np.float32)

    s = gamma / np.sqrt(var + EPS)
    bsh = beta - mean * s
    wd = wf[:, 0] - wf[:, 1]
    bd = float(bf[0] - bf[1])

    c = {}
    Wx = np.zeros((D, 49), np.float32)
    for i in range(STACK):
        for u in range(UNITS):
            Wx[:, 12 * u + i] = ws[i][4 * i:, u] * s[i, u]
    Wx[:, 48] = wd[48:]
    c["wx"] = _bf16(Wx)

    # A chunks: stage i, chunk cc = source stages {2cc, 2cc+1}
    for i in range(1, STACK):
        for cc in range(i // 2):
            M = np.zeros((128, 64), np.float32)
            for jj in range(2):
                j = 2 * cc + jj
                for v in range(4):
                    for u in range(4):
                        val = ws[i][4 * (i - 1 - j) + v, u] * s[i, u]
                        M[64 * jj + 16 * v:64 * jj + 16 * v + 16, 16 * u:16 * u + 16] \
                            [np.arange(16), np.arange(16)] = val
            c[f"a_{i}_{cc}"] = _bf16(M)
        if i % 2 == 1:
            j = i - 1
            M = np.zeros((64, 64), np.float32)
            for v in range(4):
                for u in range(4):
                    val = ws[i][4 * (i - 1 - j) + v, u] * s[i, u]
                    M[16 * v:16 * v + 16, 16 * u:16 * u + 16][np.arange(16), np.arange(16)] = val
            c[f"ap_{i}"] = _bf16(M)

    for cc in range(6):
        M = np.zeros((128, 64), np.float32)
        for jj in range(2):
            j = 2 * cc + jj
            for v in range(4):
                val = wd[4 * (11 - j) + v]
                for u in range(4):
                    M[64 * jj + 16 * v:64 * jj + 16 * v + 16, 16 * u:16 * u + 16] \
                        [np.arange(16), np.arange(16)] = val
        c[f"wd_{cc}"] = _bf16(M)

    c["s_id"] = _bf16(np.eye(64, dtype=np.float32))
    S_d = np.zeros((64, 64), np.float32)
    for t in range(T):
        for u in range(4):
            S_d[t, 16 * u + t] = 1.0
    c["s_d"] = _bf16(S_d)

    B = np.zeros((64, STACK), np.float32)
    for i in range(STACK):
        for u in range(4):
            B[16 * u:16 * u + 16, i] = bsh[i, u]
    c["bias"] = B
    c["bd"] = bd
    return c


def build_kernel(ctx, tc, outs, ins, *, bd):
    import concourse.mybir as mybir

    nc = tc.nc
    f32 = mybir.dt.float32
    bf16 = mybir.dt.bfloat16
    ACT = mybir.ActivationFunctionType
    ALU = mybir.AluOpType

    x_ap = ins["x"]
    out_ap = outs["out"]

    const_pool = ctx.enter_context(tc.tile_pool(name="consts", bufs=1))

    def load_const(name, shape, dt=f32):
        t = const_pool.tile(list(shape), dt, tag=name, name=name)
        nc.sync.dma_start(t[:], ins[name])
        return t

    wx_sb = load_const("wx", (D, 49), bf16)
    a_sb = {}
    ap_sb = {}
    for i in range(1, STACK):
        for cc in range(i // 2):
            a_sb[(i, cc)] = load_const(f"a_{i}_{cc}", (128, 64), bf16)
        if i % 2 == 1:
            ap_sb[i] = load_const(f"ap_{i}", (64, 64), bf16)
    wd_sb = [load_const(f"wd_{cc}", (128, 64), bf16) for cc in range(6)]
    sid_sb = load_const("s_id", (64, 64), bf16)
    sd_sb = load_const("s_d", (64, 64), bf16)
    bias_sb = load_const("bias", (64, STACK))

    xt_pool = ctx.enter_context(tc.tile_pool(name="xt", bufs=3))
    cx_pool = ctx.enter_context(tc.tile_pool(name="cx", bufs=3))
    cxf_pool = ctx.enter_context(tc.tile_pool(name="cxf", bufs=2))
    cxfd_pool = ctx.enter_context(tc.tile_pool(name="cxfd", bufs=2))
    y2_pool = ctx.enter_context(tc.tile_pool(name="y2", bufs=2))
    out_pool = ctx.enter_context(tc.tile_pool(name="outsb", bufs=2))

    pcx_pool = ctx.enter_context(tc.tile_pool(name="pcx", bufs=2, space="PSUM"))
    z_pool = ctx.enter_context(tc.tile_pool(name="z", bufs=2, space="PSUM"))

    # Per-group state (tiles), created lazily by the pipeline below.
    state = {}

    def start_group(g):
        cxF = cxf_pool.tile([64, STACK * NB], bf16, tag="cxF", name="cxF")
        cxFd = cxfd_pool.tile([64, NB], bf16, tag="cxFd", name="cxFd")
        # rows 16..64 of cxFd feed the K=64 d-inject; zero everything first
        # (scatters then overwrite rows 0..15)
        nc.vector.memset(cxFd[:], 0.0)
        y2 = [y2_pool.tile([128, NB], bf16, tag=f"y2c{cc}", name=f"y2c{cc}")
              for cc in range(6)]
        state[g] = (cxF, cxFd, y2)

    def emit_xtile(g, t):
        cxF, cxFd, y2 = state[g]
        r0 = g * G_ROWS + t * NB
        xt = xt_pool.tile([NPART, NB], bf16, tag="xt")
        eng = nc.sync if t % 2 == 0 else nc.scalar
        eng.dma_start(xt[:], x_ap[r0:r0 + NB, :], transpose=True)
        cx = cx_pool.tile([49, NB], bf16, tag="cx")
        for rc in range(NH):
            pcx = pcx_pool.tile([49, 1024], f32, tag="pcx")
            for h in range(2):
                nc.tensor.matmul(
                    pcx[:, h * 512:(h + 1) * 512], wx_sb[:],
                    xt[:, rc * 1024 + h * 512: rc * 1024 + (h + 1) * 512],
                    start=True, stop=True,
                )
            dst = cx[:, rc * 1024:(rc + 1) * 1024]
            if (t + rc) % 2 == 0:
                nc.scalar.activation(dst, pcx[:], ACT.Copy)
            else:
                nc.vector.tensor_copy(dst, pcx[:])
        # one-DMA scatter: rows 12u+i -> cxF[16u+t, block i]
        ed = cxF[:].rearrange("(u s) (i n) -> u s i n", u=4, i=STACK)[:, t]
        nc.gpsimd.dma_start(ed, cx[0:48, :])
        nc.scalar.dma_start(cxFd[t:t + 1, :], cx[48:49, :])

    def emit_stage(g, i):
        cxF, cxFd, y2 = state[g]
        z = z_pool.tile([128, 1024], f32, tag="z")

        def slices(rc, cb):
            zsl = z[cb * 64:(cb + 1) * 64, rc * 512:(rc + 1) * 512]
            ysl = slice(rc * 1024 + cb * 512, rc * 1024 + (cb + 1) * 512)
            return zsl, ysl

        ncc = i // 2
        has_part = (i % 2 == 1)
        # stationary-major emission: 4 matmuls (rc x cb) per weight load
        for rc in range(NH):
            for cb in range(2):
                zsl, ysl = slices(rc, cb)
                csl = slice(i * NB + rc * 1024 + cb * 512,
                            i * NB + rc * 1024 + (cb + 1) * 512)
                nc.tensor.matmul(zsl, sid_sb[:], cxF[0:64, csl],
                                 start=True, stop=(ncc == 0 and not has_part))
        if has_part:
            for rc in range(NH):
                for cb in range(2):
                    zsl, ysl = slices(rc, cb)
                    nc.tensor.matmul(zsl, ap_sb[i][:], y2[i // 2][0:64, ysl],
                                     start=False, stop=(ncc == 0))
        for cc in range(ncc):
            for rc in range(NH):
                for cb in range(2):
                    zsl, ysl = slices(rc, cb)
                    nc.tensor.matmul(zsl, a_sb[(i, cc)][:], y2[cc][:, ysl],
                                     start=False, stop=(cc == ncc - 1))
        # relu + bias -> y2 slice (per cb; strided dst over rc)
        ch, half = i // 2, 64 * (i % 2)
        for cb in range(2):
            src = z[cb * 64:(cb + 1) * 64, :].rearrange("p (rc n) -> p rc n", rc=NH)
            dst = y2[ch][half:half + 64, :].rearrange(
                "p (rc c n) -> p rc c n", rc=NH, c=2)[:, :, cb, :]
            if (i + cb) % 2 == 0:
                nc.scalar.activation(dst, src, ACT.Relu, bias=bias_sb[:, i:i + 1])
            else:
                nc.vector.tensor_scalar(dst, src, bias_sb[:, i:i + 1], 0.0,
                                        ALU.add, ALU.max)

    def emit_tail(g):
        cxF, cxFd, y2 = state[g]
        pd = z_pool.tile([128, 1024], f32, tag="z")
        for rc in range(NH):
            for cb in range(2):
                psl = pd[cb * 64:(cb + 1) * 64, rc * 512:(rc + 1) * 512]
                dsl = slice(rc * 1024 + cb * 512, rc * 1024 + (cb + 1) * 512)
                nc.tensor.matmul(psl, sd_sb[:], cxFd[0:64, dsl],
                                 start=True, stop=False)
        for cc in range(6):
            for rc in range(NH):
                for cb in range(2):
                    psl = pd[cb * 64:(cb + 1) * 64, rc * 512:(rc + 1) * 512]
                    ysl = slice(rc * 1024 + cb * 512, rc * 1024 + (cb + 1) * 512)
                    nc.tensor.matmul(psl, wd_sb[cc][:], y2[cc][:, ysl],
                                     start=False, stop=(cc == 5))
        outsb = out_pool.tile([128, NB], f32, tag="outsb")
        o4 = outsb[:].rearrange("p (rc n two) -> p rc n two", rc=NH, two=2)
        psrc = pd[:].rearrange("p (rc n) -> p rc n", rc=NH)
        nc.scalar.activation(o4[:, :, :, 0], psrc, ACT.Sigmoid, bias=float(bd))
        nc.scalar.activation(o4[:, :, :, 1], psrc, ACT.Sigmoid, bias=float(-bd),
                             scale=-1.0)
        og = out_ap[g * G_ROWS:(g + 1) * G_ROWS, :].rearrange(
            "(t rc c n) two -> c t rc (n two)", rc=NH, c=2, n=512)
        for cb in range(2):
            osrc = outsb[cb * 64:cb * 64 + T, :].rearrange("p (rc f) -> p rc f", rc=NH)
            nc.gpsimd.dma_start(og[cb], osrc)

    # Software pipeline: group g's recurrence interleaves group g+1's x-tiles
    # so the PE never drains (keeps the HAM clock warm).
    start_group(0)
    for t in range(T):
        emit_xtile(0, t)
    for g in range(GROUPS):
        if g + 1 < GROUPS:
            start_group(g + 1)
        emitted = 0
        for i in range(STACK):
            emit_stage(g, i)
            if g + 1 < GROUPS:
                want = (i + 1) * T // STACK
                while emitted < want:
                    emit_xtile(g + 1, emitted)
                    emitted += 1
        emit_tail(g)


# ---------------------------------------------------------------------------
# Self-contained entry point: kernel(**inputs) -> [500000, 2] float32
# ---------------------------------------------------------------------------

import sys as _sys
if '/opt/trn_rl_repo' not in _sys.path:
    _sys.path.insert(0, '/opt/trn_rl_repo')

B_FULL = 500000
N_CORES = 8
CORE_ROWS = GROUPS * G_ROWS                      # 65536
B_PAD = CORE_ROWS * N_CORES                      # 524288

_CACHE = {}


def _build_nc(const_shapes, bd):
    from contextlib import ExitStack
    import concourse.mybir as mybir
    from concourse import bacc
    import concourse.tile as tile

    nc = bacc.Bacc("TRN2", target_bir_lowering=False, debug=False,
                   num_devices=N_CORES)
    ins = {}
    ins["x"] = nc.dram_tensor("x", [CORE_ROWS, D], mybir.dt.bfloat16,
                              kind="ExternalInput").ap()
    for name, shp, npdt in const_shapes:
        dt = mybir.dt.bfloat16 if npdt == 'bfloat16' else mybir.dt.float32
        ins[name] = nc.dram_tensor(name, list(shp), dt,
                                   kind="ExternalInput").ap()
    outs = {"out": nc.dram_tensor("out", [CORE_ROWS, 2], mybir.dt.float32,
                                  kind="ExternalOutput").ap()}
    with tile.TileContext(nc) as tc:
        with ExitStack() as ctx:
            build_kernel(ctx, tc, outs, ins, bd=bd)
    nc.compile()
    return nc


def kernel(**inputs):
    import numpy as np
    import ml_dtypes
    from concourse.bass_utils import run_bass_kernel_spmd

    consts = prep_consts(inputs)
    bd = consts.pop("bd")
    x = np.asarray(inputs["x"], dtype=np.float32)
    assert x.shape == (B_FULL, D)
    xp = np.zeros((B_PAD, D), ml_dtypes.bfloat16)
    xp[:B_FULL] = x.astype(ml_dtypes.bfloat16)

    key = "nc"
    if key not in _CACHE:
        shapes = tuple((k, v.shape, str(v.dtype)) for k, v in consts.items())
        _CACHE[key] = _build_nc(shapes, bd)
    nc = _CACHE[key]

    in_maps = []
    for c in range(N_CORES):
        m = {"x": xp[c * CORE_ROWS:(c + 1) * CORE_ROWS]}
        m.update(consts)
        in_maps.append(m)
    res = run_bass_kernel_spmd(nc, in_maps, core_ids=list(range(N_CORES)))
    out = np.concatenate([res.results[c]["out"] for c in range(N_CORES)], axis=0)
    return out[:B_FULL]
